# revision 1
# baseline (speedup 1.0000x reference)
"""BankedLinear (MoE-style banked linear) Trainium2 kernel.

Reference computation (per token t, with k=2 selected banks):
    out[t] = sum_k prob[t,k] * (x[t] @ W[sel[t,k]] + bias[sel[t,k]])

Strategy (expert-parallel over 8 NeuronCores):
  - Core c owns banks [8c, 8c+8).  Its weight slab (8 x 512 x 512 = 8 MB of
    fp32 information) is the dominant, unavoidable HBM traffic; each bank is
    read exactly once system-wide.
  - Host routes token-bank pairs to cores by selected bank, pre-scales each
    gathered token row by its probability, transposes to [in_feature, slot],
    and pads to CAP=32 slots per bank.
  - Precision/speed: fp32 matmul runs at 1/4 rate on the PE and bf16 at full
    rate, so both x and W are split hi/lo into two bf16 halves on the host
    (same total bytes as fp32) and each bank's product is computed as
    xh@wh + xh@wl + xl@wh accumulated in fp32 PSUM (~1e-6 rel error).
  - All arrays are pre-swizzled on the host into SBUF layout so every DMA is
    a single large contiguous 2D transfer.
  - Bias is folded in on the host (one gather + multiply-add over 1024
    pairs); host scatter-adds the per-pair device results into the output.

Fixed shapes: B=2, T=256, K=2, IN=OUT=512, NB=64 banks, 8 cores.
Capacity: 32 slots/bank (binomial mean 16, sd ~4; overflow pairs — none for
realistic routing — are handled exactly on the host as a fallback).
"""

import numpy as np
from contextlib import ExitStack

B, T, KSEL = 2, 256, 2
IN, OUT, NB = 512, 512, 64
NCORES = 8
BPC = NB // NCORES          # banks per core = 8
CAP = 32                    # padded token slots per bank
SLOTS = BPC * CAP           # 256 dispatch rows per core
PCHUNK = 128                # contraction chunk (SBUF partition dim)
KC = IN // PCHUNK           # 4 contraction chunks
GROUPS = SLOTS // 128       # output row groups of 128

_cache = {}


def _build_nc():
    """Build the Bass/Tile program (one SPMD NeuronCore program)."""
    import concourse.tile as tile
    import concourse.mybir as mybir
    from concourse import bacc

    f32 = mybir.dt.float32
    bf16 = mybir.dt.bfloat16
    nc = bacc.Bacc("TRN2", target_bir_lowering=False, debug=False,
                   num_devices=NCORES)
    # host-pre-swizzled SBUF layouts: partition dim first, contiguous free dim
    xth = nc.dram_tensor("xth", [PCHUNK, KC * SLOTS], bf16,
                         kind="ExternalInput").ap()
    xtl = nc.dram_tensor("xtl", [PCHUNK, KC * SLOTS], bf16,
                         kind="ExternalInput").ap()
    wh = nc.dram_tensor("wh", [BPC, PCHUNK, KC * OUT], bf16,
                        kind="ExternalInput").ap()
    wl = nc.dram_tensor("wl", [BPC, PCHUNK, KC * OUT], bf16,
                        kind="ExternalInput").ap()
    y = nc.dram_tensor("y", [SLOTS, OUT], f32, kind="ExternalOutput").ap()

    from concourse.tile import add_dep_helper

    def chain(dep_chain, binst, reason):
        # pin scheduler order: binst depends on the previous link
        if dep_chain:
            add_dep_helper(binst.ins, dep_chain[-1].ins, sync=False,
                           reason=reason)
        dep_chain.append(binst)

    KH = 2                      # kc chunks per weight DMA (256KB granularity)
    with tile.TileContext(nc) as tc:
        with ExitStack() as ctx:
            xpool = ctx.enter_context(tc.tile_pool(name="xp", bufs=2))
            wpool = ctx.enter_context(
                tc.tile_pool(name="wp", bufs=2 * BPC * KC // KH))
            ypool = ctx.enter_context(tc.tile_pool(name="yp", bufs=GROUPS))
            pspool = ctx.enter_context(
                tc.tile_pool(name="ps", bufs=3, space="PSUM"))

            # token dispatch first on the sync ring: every matmul needs it,
            # so it must land before the weight stream floods HBM
            xh_sb = xpool.tile([PCHUNK, KC * SLOTS], bf16, tag="xh")
            xl_sb = xpool.tile([PCHUNK, KC * SLOTS], bf16, tag="xl")


            ysbs = []
            for g in range(GROUPS):
                ysb_g = ypool.tile([128, OUT], f32, tag="y")
                ysbs.append(ysb_g)

            wq = []    # sync-ring DMA chain (keeps FIFO = compute order)
            mq = []    # PE matmul chain (keeps bank order = arrival order)
            chain(wq, nc.sync.dma_start(xh_sb[:], xth[:]), "xt first")
            chain(wq, nc.sync.dma_start(xl_sb[:], xtl[:]), "xt first")

            # Banks processed in pairs. The even bank computes in PE column
            # group 0, the odd bank in column group 1 (tile_position), so
            # their matmuls overlap in the array. Each bank accumulates in
            # its OWN psum bank (separate tiles) so the per-bank start=True
            # has_written clear cannot disturb its neighbour.
            for p in range(BPC // 2):
                whs, wls = [[], []], [[], []]
                for q in range(2):
                    j = 2 * p + q
                    for kh in range(KC // KH):
                        ks = slice(kh * KH * OUT, (kh + 1) * KH * OUT)
                        wh_t = wpool.tile([PCHUNK, KH * OUT], bf16, tag="w")
                        chain(wq, nc.sync.dma_start(wh_t[:], wh[j, :, ks]),
                              "weight ring order")
                        whs[q].append(wh_t)
                        wl_t = wpool.tile([PCHUNK, KH * OUT], bf16, tag="w")
                        chain(wq, nc.sync.dma_start(wl_t[:], wl[j, :, ks]),
                              "weight ring order")
                        wls[q].append(wl_t)

                psA = pspool.tile([CAP, OUT], f32, tag="psA")
                psB = pspool.tile([2 * CAP, OUT], f32, tag="psB")
                outs = (psA[:], psB[CAP:2 * CAP, :])
                nmm = 3 * KC
                i = 0
                first_mm = None
                for kc in range(KC):
                    kh, ko = divmod(kc, KH)
                    ws = slice(ko * OUT, (ko + 1) * OUT)
                    for term in range(3):
                        a_sb = xh_sb if term < 2 else xl_sb
                        for q in range(2):
                            j = 2 * p + q
                            xs = slice(kc * SLOTS + j * CAP,
                                       kc * SLOTS + (j + 1) * CAP)
                            b_t = whs[q][kh] if term != 1 else wls[q][kh]
                            mm = nc.tensor.matmul(
                                outs[q], a_sb[:, xs], b_t[:, ws],
                                start=(i < 2), stop=(i >= 2 * nmm - 2),
                                tile_position=(0, q * CAP),
                                skip_group_check=True)
                            if first_mm is None:
                                first_mm = mm
                                chain(mq, mm, "pair compute order")
                            i += 1
                g, gq = divmod(p, 2)
                nc.vector.tensor_copy(
                    ysbs[g][gq * 2 * CAP:gq * 2 * CAP + CAP, :], psA[:])
                nc.vector.tensor_copy(
                    ysbs[g][gq * 2 * CAP + CAP:(gq + 1) * 2 * CAP, :],
                    psB[CAP:2 * CAP, :])

            # output stores on the sync ring after all weights (ring is free
            # by then; HWDGE has lower first-byte latency than SWDGE), at
            # 2-bank granularity so the last store only waits on the last
            # two banks' copies
            for h in range(2 * GROUPS):
                g, hq = divmod(h, 2)
                chain(wq, nc.sync.dma_start(
                    y[h * 64:(h + 1) * 64, :],
                    ysbs[g][hq * 64:(hq + 1) * 64, :]), "y after weights")
    nc.compile()
    return nc


def _get_nc():
    if "nc" not in _cache:
        _cache["nc"] = _build_nc()
    return _cache["nc"]


def _split_hilo(a32):
    """fp32 array -> (hi, lo) bf16 halves with a32 ~= hi + lo."""
    import ml_dtypes
    bf = ml_dtypes.bfloat16
    hi = a32.astype(bf)
    lo = (a32 - hi.astype(np.float32)).astype(bf)
    return hi, lo


def _swizzle_x(xt):
    """[IN, SLOTS] -> [128, KC*SLOTS] with free index (kc, slot)."""
    return np.ascontiguousarray(
        xt.reshape(KC, PCHUNK, SLOTS).transpose(1, 0, 2).reshape(
            PCHUNK, KC * SLOTS))


def _swizzle_w(w):
    """[BPC, IN, OUT] -> [BPC, 128, KC*OUT] with free index (kc, out)."""
    return np.ascontiguousarray(
        w.reshape(BPC, KC, PCHUNK, OUT).transpose(0, 2, 1, 3).reshape(
            BPC, PCHUNK, KC * OUT))


def _route(X, sel, prob):
    """Group token-bank pairs by bank, build per-core dispatch arrays.

    Returns (in_maps, slot_tok [NCORES,SLOTS] int64 (-1=pad), overflow list
    of (token, bank, prob))."""
    NT = X.shape[0]
    pair_tok = np.repeat(np.arange(NT, dtype=np.int64), KSEL)
    pair_bank = sel.reshape(-1)
    pair_p = prob.reshape(-1)

    order = np.argsort(pair_bank, kind="stable")
    counts = np.bincount(pair_bank, minlength=NB)
    starts = np.concatenate(([0], np.cumsum(counts)))

    slot_tok = np.full((NCORES, SLOTS), -1, dtype=np.int64)
    slot_p = np.zeros((NCORES, SLOTS), dtype=np.float32)
    overflow = []
    for b in range(NB):
        c, j = divmod(b, BPC)
        s0, s1 = starts[b], starts[b + 1]
        take = min(s1 - s0, CAP)
        idx = order[s0:s0 + take]
        slot_tok[c, j * CAP: j * CAP + take] = pair_tok[idx]
        slot_p[c, j * CAP: j * CAP + take] = pair_p[idx]
        for i in order[s0 + take:s1]:
            overflow.append((int(pair_tok[i]), b, float(pair_p[i])))
    return slot_tok, slot_p, overflow


def _combine(ys, slot_tok, X, sel, prob, weights, bias, overflow):
    NT = X.shape[0]
    out = np.zeros((NT, OUT), dtype=np.float32)
    for c in range(NCORES):
        tok = slot_tok[c]
        valid = tok >= 0
        np.add.at(out, tok[valid], ys[c][valid])
    # bias term for every pair (device computes x @ W only)
    for k in range(KSEL):
        out += prob[:, k, None] * bias[sel[:, k]]
    # exact host fallback for capacity-overflow pairs (expected: none)
    for t, b, p in overflow:
        out[t] += p * (X[t] @ weights[b])
    return out


def _run_device(in_maps, trace=False, **kwargs):
    from concourse.bass_utils import run_bass_kernel_spmd
    return run_bass_kernel_spmd(_get_nc(), in_maps,
                                core_ids=list(range(NCORES)),
                                trace=trace, **kwargs)


def kernel(_trace=False, _bass_results=None, **inputs):
    tensor = np.asarray(inputs["tensor"], dtype=np.float32)
    sel = np.asarray(inputs["bank_selections"]).astype(np.int64)
    prob = np.asarray(inputs["bank_probabilities"], dtype=np.float32)
    weights = np.asarray(inputs["weights"], dtype=np.float32)
    bias = np.asarray(inputs["bias"], dtype=np.float32)

    NT = tensor.shape[0] * tensor.shape[1]
    X = tensor.reshape(NT, IN)
    sel2 = sel.reshape(NT, KSEL)
    prob2 = prob.reshape(NT, KSEL)

    slot_tok, slot_p, overflow = _route(X, sel2, prob2)

    in_maps = []
    for c in range(NCORES):
        tok = slot_tok[c]
        rows = X[np.where(tok >= 0, tok, 0)] * slot_p[c][:, None]
        xt = np.ascontiguousarray(rows.T)              # [IN, SLOTS] fp32
        xh, xl = _split_hilo(xt)
        w32 = weights[c * BPC:(c + 1) * BPC]           # (8, 512, 512) fp32
        wwh, wwl = _split_hilo(w32)
        in_maps.append({
            "xth": _swizzle_x(xh), "xtl": _swizzle_x(xl),
            "wh": _swizzle_w(wwh), "wl": _swizzle_w(wwl),
        })

    res = _run_device(in_maps, trace=_trace)
    if _bass_results is not None:
        _bass_results.append(res)
    ys = [res.results[c]["y"] for c in range(NCORES)]

    out = _combine(ys, slot_tok, X, sel2, prob2, weights, bias, overflow)
    return out.reshape(tensor.shape[0], tensor.shape[1], OUT)



# revision 3
# speedup vs baseline: 1.5922x; 1.5922x over previous
"""BankedLinear (MoE-style banked linear) Trainium2 kernel.

Reference computation (per token t, with k=2 selected banks):
    out[t] = sum_k prob[t,k] * (x[t] @ W[sel[t,k]] + bias[sel[t,k]])

Strategy (expert-parallel over 8 NeuronCores):
  - Core c owns banks [8c, 8c+8).  Its weight slab dominates HBM traffic;
    each bank is read exactly once system-wide.
  - Host routes token-bank pairs to cores by selected bank, pre-scales each
    gathered token row by its probability, transposes to [in_feature, slot],
    and pads to CAP=32 slots per bank.
  - Precision: the harness gate is rel_err < 2e-2, so x and W are cast to
    plain bf16 (measured end-to-end error ~2e-3).  This halves the weight
    stream vs an fp32-accurate hi/lo split (4 MB/core instead of 8 MB) and
    needs one matmul per (bank, k-chunk) instead of three.
  - Weights are host-swizzled to a single [128, BPC*KC*OUT] slab whose free
    index is (bank, kc, out): any span of banks is one contiguous 2D DMA.
  - DMA plan: x dispatch then the 8 per-bank weight DMAs (512 KB each)
    stream FIFO on the sync (SP HWDGE) ring; per-pair output stores go on
    the scalar (ACT HWDGE) ring so their descriptor-generation waits never
    stall the weight stream.
  - Bias is folded in on the host (one gather + multiply-add over 1024
    pairs); host scatter-adds the per-pair device results into the output.

Fixed shapes: B=2, T=256, K=2, IN=OUT=512, NB=64 banks, 8 cores.
Capacity: 32 slots/bank (binomial mean 16, sd ~4; overflow pairs — none for
realistic routing — are handled exactly on the host as a fallback).
"""

import numpy as np
from contextlib import ExitStack

B, T, KSEL = 2, 256, 2
IN, OUT, NB = 512, 512, 64
NCORES = 8
BPC = NB // NCORES          # banks per core = 8
CAP = 32                    # padded token slots per bank
SLOTS = BPC * CAP           # 256 dispatch rows per core
PCHUNK = 128                # contraction chunk (SBUF partition dim)
KC = IN // PCHUNK           # 4 contraction chunks
GROUPS = SLOTS // 128       # output row groups of 128

_cache = {}


def _build_nc():
    """Build the Bass/Tile program (one SPMD NeuronCore program)."""
    import concourse.tile as tile
    import concourse.mybir as mybir
    from concourse import bacc

    f32 = mybir.dt.float32
    bf16 = mybir.dt.bfloat16
    nc = bacc.Bacc("TRN2", target_bir_lowering=False, debug=False,
                   num_devices=NCORES)
    # host-pre-swizzled SBUF layouts: partition dim first, contiguous free dim
    xt = nc.dram_tensor("xt", [PCHUNK, KC * SLOTS], bf16,
                        kind="ExternalInput").ap()
    w = nc.dram_tensor("w", [PCHUNK, BPC * KC * OUT], bf16,
                       kind="ExternalInput").ap()
    y = nc.dram_tensor("y", [SLOTS, OUT], bf16, kind="ExternalOutput").ap()

    from concourse.tile import add_dep_helper

    def chain(dep_chain, binst, reason):
        # pin scheduler order: binst depends on the previous link
        if dep_chain:
            add_dep_helper(binst.ins, dep_chain[-1].ins, sync=False,
                           reason=reason)
        dep_chain.append(binst)

    with tile.TileContext(nc) as tc:
        with ExitStack() as ctx:
            xpool = ctx.enter_context(tc.tile_pool(name="xp", bufs=1))
            wpool = ctx.enter_context(tc.tile_pool(name="wp", bufs=BPC))
            ypool = ctx.enter_context(tc.tile_pool(name="yp", bufs=GROUPS))
            pspool = ctx.enter_context(
                tc.tile_pool(name="ps", bufs=3, space="PSUM"))

            # token dispatch first on the sync ring: every matmul needs it,
            # so it must land before the weight stream floods HBM
            x_sb = xpool.tile([PCHUNK, KC * SLOTS], bf16, tag="x")

            wq = []    # sync-ring DMA chain (keeps FIFO = compute order)
            mq = []    # PE matmul chain (keeps bank order = arrival order)
            sq = []    # scalar-ring store chain
            chain(wq, nc.sync.dma_start(x_sb[:], xt[:]), "xt first")

            wts = []
            for j in range(BPC):
                w_t = wpool.tile([PCHUNK, KC * OUT], bf16, tag="w")
                chain(wq, nc.sync.dma_start(
                    w_t[:], w[:, j * KC * OUT:(j + 1) * KC * OUT]),
                    "weight ring order")
                wts.append(w_t)

            ysbs = [ypool.tile([128, OUT], bf16, tag="y", name=f"ysb{g}")
                    for g in range(GROUPS)]

            # Banks processed in pairs. The even bank computes in PE column
            # group 0, the odd bank in column group 1 (tile_position), so
            # their matmuls overlap in the array. Each bank accumulates in
            # its OWN psum bank (separate tiles) so the per-bank start=True
            # has_written clear cannot disturb its neighbour.
            for p in range(BPC // 2):
                psA = pspool.tile([CAP, OUT], f32, tag="psA")
                psB = pspool.tile([2 * CAP, OUT], f32, tag="psB")
                outs = (psA[:], psB[CAP:2 * CAP, :])
                first_mm = None
                for kc in range(KC):
                    ws = slice(kc * OUT, (kc + 1) * OUT)
                    for q in range(2):
                        j = 2 * p + q
                        xs = slice(kc * SLOTS + j * CAP,
                                   kc * SLOTS + (j + 1) * CAP)
                        mm = nc.tensor.matmul(
                            outs[q], x_sb[:, xs], wts[j][:, ws],
                            start=(kc == 0), stop=(kc == KC - 1),
                            tile_position=(0, q * CAP),
                            skip_group_check=True)
                        if first_mm is None:
                            first_mm = mm
                            chain(mq, mm, "pair compute order")
                g, gq = divmod(p, 2)
                r0 = gq * 2 * CAP
                nc.vector.tensor_copy(ysbs[g][r0:r0 + CAP, :], psA[:])
                nc.vector.tensor_copy(ysbs[g][r0 + CAP:r0 + 2 * CAP, :],
                                      psB[CAP:2 * CAP, :])
                # store this pair's 64 rows on the ACT ring as soon as the
                # copies land; the sync ring keeps streaming weights
                chain(sq, nc.scalar.dma_start(
                    y[p * 2 * CAP:(p + 1) * 2 * CAP, :],
                    ysbs[g][r0:r0 + 2 * CAP, :]), "y store order")
    nc.compile()
    return nc


def _get_nc():
    if "nc" not in _cache:
        _cache["nc"] = _build_nc()
    return _cache["nc"]


def _bf16(a32):
    import ml_dtypes
    return a32.astype(ml_dtypes.bfloat16)


def _swizzle_x(xt):
    """[IN, SLOTS] -> [128, KC*SLOTS] with free index (kc, slot)."""
    return np.ascontiguousarray(
        xt.reshape(KC, PCHUNK, SLOTS).transpose(1, 0, 2).reshape(
            PCHUNK, KC * SLOTS))


def _swizzle_w(w):
    """[BPC, IN, OUT] -> [128, BPC*KC*OUT] with free index (bank, kc, out)."""
    return np.ascontiguousarray(
        w.reshape(BPC, KC, PCHUNK, OUT).transpose(2, 0, 1, 3).reshape(
            PCHUNK, BPC * KC * OUT))


def _route(X, sel, prob):
    """Group token-bank pairs by bank, build per-core dispatch arrays.

    Returns (slot_tok [NCORES,SLOTS] int64 (-1=pad), slot_p, overflow list
    of (token, bank, prob))."""
    NT = X.shape[0]
    pair_tok = np.repeat(np.arange(NT, dtype=np.int64), KSEL)
    pair_bank = sel.reshape(-1)
    pair_p = prob.reshape(-1)

    order = np.argsort(pair_bank, kind="stable")
    counts = np.bincount(pair_bank, minlength=NB)
    starts = np.concatenate(([0], np.cumsum(counts)))

    slot_tok = np.full((NCORES, SLOTS), -1, dtype=np.int64)
    slot_p = np.zeros((NCORES, SLOTS), dtype=np.float32)
    overflow = []
    for b in range(NB):
        c, j = divmod(b, BPC)
        s0, s1 = starts[b], starts[b + 1]
        take = min(s1 - s0, CAP)
        idx = order[s0:s0 + take]
        slot_tok[c, j * CAP: j * CAP + take] = pair_tok[idx]
        slot_p[c, j * CAP: j * CAP + take] = pair_p[idx]
        for i in order[s0 + take:s1]:
            overflow.append((int(pair_tok[i]), b, float(pair_p[i])))
    return slot_tok, slot_p, overflow


def _combine(ys, slot_tok, X, sel, prob, weights, bias, overflow):
    NT = X.shape[0]
    out = np.zeros((NT, OUT), dtype=np.float32)
    for c in range(NCORES):
        tok = slot_tok[c]
        valid = tok >= 0
        np.add.at(out, tok[valid], ys[c][valid].astype(np.float32))
    # bias term for every pair (device computes x @ W only)
    for k in range(KSEL):
        out += prob[:, k, None] * bias[sel[:, k]]
    # exact host fallback for capacity-overflow pairs (expected: none)
    for t, b, p in overflow:
        out[t] += p * (X[t] @ weights[b])
    return out


def _run_device(in_maps, trace=False, **kwargs):
    from concourse.bass_utils import run_bass_kernel_spmd
    return run_bass_kernel_spmd(_get_nc(), in_maps,
                                core_ids=list(range(NCORES)),
                                trace=trace, **kwargs)


def kernel(_trace=False, _bass_results=None, **inputs):
    tensor = np.asarray(inputs["tensor"], dtype=np.float32)
    sel = np.asarray(inputs["bank_selections"]).astype(np.int64)
    prob = np.asarray(inputs["bank_probabilities"], dtype=np.float32)
    weights = np.asarray(inputs["weights"], dtype=np.float32)
    bias = np.asarray(inputs["bias"], dtype=np.float32)

    NT = tensor.shape[0] * tensor.shape[1]
    X = tensor.reshape(NT, IN)
    sel2 = sel.reshape(NT, KSEL)
    prob2 = prob.reshape(NT, KSEL)

    slot_tok, slot_p, overflow = _route(X, sel2, prob2)

    in_maps = []
    for c in range(NCORES):
        tok = slot_tok[c]
        rows = X[np.where(tok >= 0, tok, 0)] * slot_p[c][:, None]
        xt = np.ascontiguousarray(rows.T)              # [IN, SLOTS] fp32
        w32 = weights[c * BPC:(c + 1) * BPC]           # (8, 512, 512) fp32
        in_maps.append({
            "xt": _bf16(_swizzle_x(xt)),
            "w": _bf16(_swizzle_w(w32)),
        })

    res = _run_device(in_maps, trace=_trace)
    if _bass_results is not None:
        _bass_results.append(res)
    ys = [res.results[c]["y"] for c in range(NCORES)]

    out = _combine(ys, slot_tok, X, sel2, prob2, weights, bias, overflow)
    return out.reshape(tensor.shape[0], tensor.shape[1], OUT)


# revision 4
# speedup vs baseline: 1.6638x; 1.0450x over previous
"""BankedLinear (MoE-style banked linear) Trainium2 kernel.

Reference computation (per token t, with k=2 selected banks):
    out[t] = sum_k prob[t,k] * (x[t] @ W[sel[t,k]] + bias[sel[t,k]])

Strategy (expert-parallel over 8 NeuronCores):
  - Core c owns banks [8c, 8c+8).  Its weight slab dominates HBM traffic;
    each bank is read exactly once system-wide (4 MB/core as bf16).
  - Host routes token-bank pairs to cores by selected bank, pre-scales each
    gathered token row by its probability, transposes to [in_feature, slot],
    and pads to CAP=32 slots per bank.
  - Precision: the harness gate is rel_err < 2e-2, so x and W are cast to
    plain bf16 (measured end-to-end error ~3e-3).  This halves the weight
    stream vs an fp32-accurate hi/lo split and needs one matmul per
    (bank, k-chunk) instead of three.
  - Weights are host-swizzled to a single [128, BPC*KC*OUT] slab whose free
    index is (bank, kc, out): any span of banks is one contiguous 2D DMA.
  - DMA plan: the 8 per-bank weight DMAs (512 KB each) stream FIFO on the
    sync (SP HWDGE) ring at ~350 GB/s; the x dispatch and the per-quad
    output stores ride the scalar (ACT HWDGE) ring so they never stall the
    weight stream.
  - PSUM: four banks share one [128, OUT] PSUM tile at partition offsets
    0/32/64/96 (tile_position column groups), so one full-width DVE cast
    and one 128 KB store cover four banks.
  - PE warm-up: the PE clock sits at 1.2 GHz until the HAM sees ~3.4 us of
    sustained activity.  A burst of dummy matmuls on a zeroed scratch tile
    warms it during the DMA head so real matmuls run at 2.4 GHz and keep
    pace with the weight stream.
  - Bias is folded in on the host (one gather + multiply-add over 1024
    pairs); host scatter-adds the per-pair device results into the output.

Fixed shapes: B=2, T=256, K=2, IN=OUT=512, NB=64 banks, 8 cores.
Capacity: 32 slots/bank (binomial mean 16, sd ~4; overflow pairs — none for
realistic routing — are handled exactly on the host as a fallback).
"""

import numpy as np
from contextlib import ExitStack

B, T, KSEL = 2, 256, 2
IN, OUT, NB = 512, 512, 64
NCORES = 8
BPC = NB // NCORES          # banks per core = 8
CAP = 32                    # padded token slots per bank
SLOTS = BPC * CAP           # 256 dispatch rows per core
PCHUNK = 128                # contraction chunk (SBUF partition dim)
KC = IN // PCHUNK           # 4 contraction chunks
QUADS = BPC // 4            # 4 banks share one PSUM tile / output store
NWARM = 10                  # dummy matmuls to lift the HAM clock gate

_cache = {}


def _build_nc():
    """Build the Bass/Tile program (one SPMD NeuronCore program)."""
    import concourse.tile as tile
    import concourse.mybir as mybir
    from concourse import bacc

    f32 = mybir.dt.float32
    bf16 = mybir.dt.bfloat16
    nc = bacc.Bacc("TRN2", target_bir_lowering=False, debug=False,
                   num_devices=NCORES)
    # host-pre-swizzled SBUF layouts: partition dim first, contiguous free dim
    xt = nc.dram_tensor("xt", [PCHUNK, KC * SLOTS], bf16,
                        kind="ExternalInput").ap()
    w = nc.dram_tensor("w", [PCHUNK, BPC * KC * OUT], bf16,
                       kind="ExternalInput").ap()
    y = nc.dram_tensor("y", [SLOTS, OUT], bf16, kind="ExternalOutput").ap()

    from concourse.tile import add_dep_helper

    def chain(dep_chain, binst, reason):
        # pin scheduler order: binst depends on the previous link
        if dep_chain:
            add_dep_helper(binst.ins, dep_chain[-1].ins, sync=False,
                           reason=reason)
        dep_chain.append(binst)

    with tile.TileContext(nc) as tc:
        with ExitStack() as ctx:
            xpool = ctx.enter_context(tc.tile_pool(name="xp", bufs=1))
            wpool = ctx.enter_context(tc.tile_pool(name="wp", bufs=BPC))
            ypool = ctx.enter_context(tc.tile_pool(name="yp", bufs=QUADS))
            pspool = ctx.enter_context(
                tc.tile_pool(name="ps", bufs=QUADS + 1, space="PSUM"))

            # x dispatch rides the scalar (ACT) ring, concurrent with the
            # weight stream on the sync (SP) ring
            x_sb = xpool.tile([PCHUNK, KC * SLOTS], bf16, tag="x")
            sq = []    # scalar-ring chain: x, then per-quad y stores
            chain(sq, nc.scalar.dma_start(x_sb[:], xt[:]), "x first on ACT")

            wq = []    # sync-ring DMA chain (keeps FIFO = compute order)
            wts = []
            for j in range(BPC):
                w_t = wpool.tile([PCHUNK, KC * OUT], bf16, tag="w",
                                 name=f"w{j}")
                chain(wq, nc.sync.dma_start(
                    w_t[:], w[:, j * KC * OUT:(j + 1) * KC * OUT]),
                    "weight ring order")
                wts.append(w_t)

            ysbs = [ypool.tile([128, OUT], bf16, tag="y", name=f"ysb{g}")
                    for g in range(QUADS)]

            # PE warm-up: dummy matmuls on a zeroed scratch tile, ordered
            # before the real matmuls.  They run during the DMA head and
            # flip the HAM clock gate from 1.2 to 2.4 GHz.
            warm_x = xpool.tile([PCHUNK, OUT], bf16, tag="wx")
            warm_ps = pspool.tile([CAP, OUT], f32, tag="wps")
            nc.gpsimd.memset(warm_x[:], 0.0)
            mq = []    # PE order chain: warm-up first, then bank order
            for i in range(NWARM):
                mm = nc.tensor.matmul(warm_ps[:], warm_x[:, :CAP],
                                      warm_x[:, :OUT],
                                      start=True, stop=True,
                                      skip_group_check=True)
                chain(mq, mm, "warmup order")

            # Four banks per quad: bank b lands in PE column group b
            # (tile_position) and PSUM partitions 32b..32b+32 of ONE shared
            # [128, OUT] PSUM tile, so a single full-width DVE cast and one
            # 128 KB store cover the whole quad.
            for qd in range(QUADS):
                ps = pspool.tile([128, OUT], f32, tag="ps", name=f"ps{qd}")
                for b in range(4):
                    j = 4 * qd + b
                    for kc in range(KC):
                        xs = slice(kc * SLOTS + j * CAP,
                                   kc * SLOTS + (j + 1) * CAP)
                        ws = slice(kc * OUT, (kc + 1) * OUT)
                        mm = nc.tensor.matmul(
                            ps[b * CAP:(b + 1) * CAP, :],
                            x_sb[:, xs], wts[j][:, ws],
                            start=(kc == 0), stop=(kc == KC - 1),
                            tile_position=(0, b * CAP),
                            skip_group_check=True)
                        if kc == 0:
                            chain(mq, mm, "bank compute order")
                nc.vector.tensor_copy(ysbs[qd][:], ps[:])
                chain(sq, nc.scalar.dma_start(
                    y[qd * 128:(qd + 1) * 128, :], ysbs[qd][:]),
                    "y store order")
    nc.compile()
    return nc


def _get_nc():
    if "nc" not in _cache:
        _cache["nc"] = _build_nc()
    return _cache["nc"]


def _bf16(a32):
    import ml_dtypes
    return a32.astype(ml_dtypes.bfloat16)


def _swizzle_x(xt):
    """[IN, SLOTS] -> [128, KC*SLOTS] with free index (kc, slot)."""
    return np.ascontiguousarray(
        xt.reshape(KC, PCHUNK, SLOTS).transpose(1, 0, 2).reshape(
            PCHUNK, KC * SLOTS))


def _swizzle_w(w):
    """[BPC, IN, OUT] -> [128, BPC*KC*OUT] with free index (bank, kc, out)."""
    return np.ascontiguousarray(
        w.reshape(BPC, KC, PCHUNK, OUT).transpose(2, 0, 1, 3).reshape(
            PCHUNK, BPC * KC * OUT))


def _route(X, sel, prob):
    """Group token-bank pairs by bank, build per-core dispatch arrays.

    Returns (slot_tok [NCORES,SLOTS] int64 (-1=pad), slot_p, overflow list
    of (token, bank, prob))."""
    NT = X.shape[0]
    pair_tok = np.repeat(np.arange(NT, dtype=np.int64), KSEL)
    pair_bank = sel.reshape(-1)
    pair_p = prob.reshape(-1)

    order = np.argsort(pair_bank, kind="stable")
    counts = np.bincount(pair_bank, minlength=NB)
    starts = np.concatenate(([0], np.cumsum(counts)))

    slot_tok = np.full((NCORES, SLOTS), -1, dtype=np.int64)
    slot_p = np.zeros((NCORES, SLOTS), dtype=np.float32)
    overflow = []
    for b in range(NB):
        c, j = divmod(b, BPC)
        s0, s1 = starts[b], starts[b + 1]
        take = min(s1 - s0, CAP)
        idx = order[s0:s0 + take]
        slot_tok[c, j * CAP: j * CAP + take] = pair_tok[idx]
        slot_p[c, j * CAP: j * CAP + take] = pair_p[idx]
        for i in order[s0 + take:s1]:
            overflow.append((int(pair_tok[i]), b, float(pair_p[i])))
    return slot_tok, slot_p, overflow


def _combine(ys, slot_tok, X, sel, prob, weights, bias, overflow):
    NT = X.shape[0]
    out = np.zeros((NT, OUT), dtype=np.float32)
    for c in range(NCORES):
        tok = slot_tok[c]
        valid = tok >= 0
        np.add.at(out, tok[valid], ys[c][valid].astype(np.float32))
    # bias term for every pair (device computes x @ W only)
    for k in range(KSEL):
        out += prob[:, k, None] * bias[sel[:, k]]
    # exact host fallback for capacity-overflow pairs (expected: none)
    for t, b, p in overflow:
        out[t] += p * (X[t] @ weights[b])
    return out


def _run_device(in_maps, trace=False, **kwargs):
    from concourse.bass_utils import run_bass_kernel_spmd
    return run_bass_kernel_spmd(_get_nc(), in_maps,
                                core_ids=list(range(NCORES)),
                                trace=trace, **kwargs)


def kernel(_trace=False, _bass_results=None, **inputs):
    tensor = np.asarray(inputs["tensor"], dtype=np.float32)
    sel = np.asarray(inputs["bank_selections"]).astype(np.int64)
    prob = np.asarray(inputs["bank_probabilities"], dtype=np.float32)
    weights = np.asarray(inputs["weights"], dtype=np.float32)
    bias = np.asarray(inputs["bias"], dtype=np.float32)

    NT = tensor.shape[0] * tensor.shape[1]
    X = tensor.reshape(NT, IN)
    sel2 = sel.reshape(NT, KSEL)
    prob2 = prob.reshape(NT, KSEL)

    slot_tok, slot_p, overflow = _route(X, sel2, prob2)

    in_maps = []
    for c in range(NCORES):
        tok = slot_tok[c]
        rows = X[np.where(tok >= 0, tok, 0)] * slot_p[c][:, None]
        xt = np.ascontiguousarray(rows.T)              # [IN, SLOTS] fp32
        w32 = weights[c * BPC:(c + 1) * BPC]           # (8, 512, 512) fp32
        in_maps.append({
            "xt": _bf16(_swizzle_x(xt)),
            "w": _bf16(_swizzle_w(w32)),
        })

    res = _run_device(in_maps, trace=_trace)
    if _bass_results is not None:
        _bass_results.append(res)
    ys = [res.results[c]["y"] for c in range(NCORES)]

    out = _combine(ys, slot_tok, X, sel2, prob2, weights, bias, overflow)
    return out.reshape(tensor.shape[0], tensor.shape[1], OUT)
